# revision 42
# baseline (speedup 1.0000x reference)
"""Trainium2 Bass kernel for nn_BindingAffinityPredictor (GNN message passing).

Strategy (8 NeuronCores, SPMD):
- Sort edges by dst; partition nodes into 8 contiguous ranges with ~equal
  edge counts. Each core owns its node range and ALL edges into it, so the
  scatter-add is core-local.
- Within a core, nodes are greedily packed into windows of <=64 nodes and
  <=1024 edges; every window gets exactly 8 edge-tiles of 128 (padded).
  Node "slots" = window*64 + pos give a fixed, data-independent layout so
  one SPMD program serves all cores.
- Math trick: msg_in @ W1 splits into P[src] + Q[dst] + ef@W1c + b1 where
  P = x@W1a, Q = x@W1b are node-level matmuls (16x fewer FLOPs than
  per-edge). The second msg linear commutes with segment_sum:
  agg = segsum(relu(h1)) @ W2 + deg*b2.
- Edge phase is all fp8e4m3 (final output is a mean over 10k nodes, so
  quantization noise averages out; measured ~1.8e-3 rel err):
  * P table (x@W1a) cast to fp8, AllGather'ed (double-buffered by layer
    parity, optionally chunk-major + per-chunk early-fired collectives),
    then dma_gather'ed per edge (2 merged 1024-idx gathers per chunk).
  * Per edge tile [128 edges]: one fp16 matmul computes
    onehot_dst.T@(Q|W1c|b1), one fp8 matmul folds +I@P[src] into the same
    PSUM. One relu per tile-PAIR ([128,512] PSUM, alternating
    scalar/vector engines) writes fp8, then one fp8 DoubleRow scatter
    matmul (0.5 cyc/row, k-subtiles = the two scat one-hots / the two
    relu outputs) per pair accumulates both tiles' segment sums in PSUM.
- Node-level matmuls run in float32r (full PE speed at N>=256, ~fp32
  precision).
- PITFALL: every rhs partition row a matmul reads must be initialized
  (0 * NaN = NaN on PE can poison PSUM).
"""

import os
import sys

sys.path.insert(0, "/opt/trn_rl_repo")

import numpy as np

N_NODES = 10000
N_EDGES = 160000
HID = 256
NBOND = 6
NATOM = 62
NLAYERS = 6
N_CORES = 8

WN = 64          # nodes per window
WT = 8           # edge tiles per window
WE = WT * 128    # padded edge slots per window

# P-table distribution knobs (must match between _plan and _build)
PFP8 = bool(int(os.environ.get("KERNEL_PFP8", "1")))     # fp8e4m3 P table
GQ = int(os.environ.get("KERNEL_GQ", "2"))               # gathers per chunk
AGCHUNK = bool(int(os.environ.get("KERNEL_AGCHUNK", "1")))  # chunked AllGather

_cache: dict = {}


def _chunk_list(slots):
    # 512-wide chunks (last one 256): narrower chunks drop the float32r
    # node matmuls below the 256-row full-speed threshold — do not shrink
    chunks = []
    c0 = 0
    while c0 < slots:
        w = min(512, slots - c0)
        chunks.append((c0, w))
        c0 += w
    return chunks


# ----------------------------------------------------------------------------
# Host-side planning: core boundaries, windows, packed per-core arrays
# ----------------------------------------------------------------------------
def _plan(edge_index: np.ndarray, edge_features: np.ndarray,
          atom_features: np.ndarray):
    src = np.asarray(edge_index[0], dtype=np.int64)
    dst = np.asarray(edge_index[1], dtype=np.int64)
    deg = np.bincount(dst, minlength=N_NODES).astype(np.int64)
    cumdeg = np.concatenate([[0], np.cumsum(deg)])  # edges with dst < n

    bounds = [0]
    for c in range(1, N_CORES):
        bounds.append(int(np.searchsorted(cumdeg, N_EDGES * c / N_CORES)))
    bounds.append(N_NODES)

    order = np.argsort(dst, kind="stable")

    # greedy windows per core
    core_windows = []  # list of list of (node_start, node_cnt, edge_cnt)
    for c in range(N_CORES):
        wins = []
        n = bounds[c]
        while n < bounds[c + 1]:
            start, cnt, ecnt = n, 0, 0
            while (n < bounds[c + 1] and cnt < WN
                   and ecnt + deg[n] <= WE):
                ecnt += deg[n]
                cnt += 1
                n += 1
            assert cnt > 0, "single node exceeds window edge capacity"
            wins.append((start, cnt, ecnt))
        core_windows.append(wins)

    nwin = max(len(w) for w in core_windows)
    nwin = (nwin + 3) // 4 * 4  # S % 256 == 0 and even gather chunks
    slots = nwin * WN
    tiles = nwin * WT
    ep = tiles * 128  # padded edge slots per core

    # slot maps
    slot_of = np.full(N_NODES, -1, dtype=np.int64)
    core_of = np.full(N_NODES, -1, dtype=np.int64)
    for c, wins in enumerate(core_windows):
        for w, (s, cnt, _e) in enumerate(wins):
            slot_of[s:s + cnt] = w * WN + np.arange(cnt)
            core_of[s:s + cnt] = c
    if AGCHUNK:
        # chunk-major p_full layout: the AllGather for slot-chunk (cc, cw)
        # concatenates the 8 cores' rows contiguously at 8*cc, so row =
        # 8*cc + core*cw + (slot - cc)
        gslot = np.full(N_NODES, -1, dtype=np.int64)
        valid = slot_of >= 0
        for cc, cw in _chunk_list(slots):
            m = valid & (slot_of >= cc) & (slot_of < cc + cw)
            gslot[m] = 8 * cc + core_of[m] * cw + (slot_of[m] - cc)
    else:
        gslot = core_of * slots + slot_of  # global padded slot id

    import ml_dtypes
    f8 = ml_dtypes.float8_e4m3fn

    per_core = []
    for c, wins in enumerate(core_windows):
        comb = np.zeros((72, ep), np.float16)
        scat = np.zeros((128, tiles * WN), f8)
        srcg = np.zeros(ep, np.int16)
        for w, (s, cnt, ecnt) in enumerate(wins):
            if ecnt == 0:
                continue
            e_ids = order[cumdeg[s]:cumdeg[s + cnt]]  # dst-sorted, in window
            # sort window's edges by src for gather locality (dst stays in
            # window; one-hots are per-edge so order within window is free)
            e_ids = e_ids[np.argsort(gslot[src[e_ids]], kind="stable")]
            j = np.arange(ecnt)
            base = w * WE
            pos = base + j
            dl = dst[e_ids] - s                      # local node pos 0..cnt-1
            comb[0:64][dl, pos] = 1.0                 # bcast one-hot
            comb[64:70, pos] = edge_features[e_ids].T.astype(np.float16)
            comb[70, pos] = 1.0                       # bias ones
            tt = j // 128
            pp = j % 128
            scat[pp, (w * WT + tt) * WN + dl] = 1.0
            srcg[pos] = gslot[src[e_ids]].astype(np.int16)
        # wrap indices for dma_gather: element i at [i % 16, i // 16],
        # replicated across the 8 groups of 16 partitions
        wrapped = srcg.reshape(-1, 16).T              # [16, ep/16]
        srcw = np.tile(wrapped, (8, 1))               # [128, ep/16]

        af = np.zeros((64, slots), np.float32)
        degs = np.zeros((1, slots), np.float16)
        mask = np.zeros((1, slots), np.float16)
        for w, (s, cnt, _e) in enumerate(wins):
            sl = slice(w * WN, w * WN + cnt)
            af[:NATOM, sl] = atom_features[s:s + cnt].T
            degs[0, sl] = deg[s:s + cnt]
            mask[0, sl] = 1.0
        per_core.append(dict(comb=comb, scat=scat.reshape(128, tiles, WN),
                             srcw=srcw, af=af, deg=degs, mask=mask))

    return dict(nwin=nwin, slots=slots, tiles=tiles, ep=ep,
                per_core=per_core, bounds=bounds, core_windows=core_windows)


def _pack_weights(inp):
    """Pack weights into the layouts the device program consumes."""
    f32 = np.float32
    L = NLAYERS
    w1 = np.asarray(inp["msg_w1"], f32)   # [L, 518, 256]
    b1 = np.asarray(inp["msg_b1"], f32)   # [L, 256]
    w2 = np.asarray(inp["msg_w2"], f32)   # [L, 256, 256]
    b2 = np.asarray(inp["msg_b2"], f32)
    u1 = np.asarray(inp["upd_w1"], f32)   # [L, 512, 256]
    ub1 = np.asarray(inp["upd_b1"], f32)
    u2 = np.asarray(inp["upd_w2"], f32)
    ub2 = np.asarray(inp["upd_b2"], f32)

    # W1AB [L, 128, 1024]: col k2*512 + 0:256 = w1a, +256:512 = w1b
    w1ab = np.zeros((L, 128, 1024), f32)
    for k2 in range(2):
        w1ab[:, :, k2 * 512:k2 * 512 + 256] = w1[:, k2 * 128:(k2 + 1) * 128, :]
        w1ab[:, :, k2 * 512 + 256:(k2 + 1) * 512] = \
            w1[:, 256 + k2 * 128:256 + (k2 + 1) * 128, :]

    # W1CB [L, 8, 256] fp16: rows 0:6 = w1c, row 6 = b1, row 7 = 0
    w1cb = np.zeros((L, 8, 256), np.float16)
    w1cb[:, 0:6] = w1[:, 512:518, :].astype(np.float16)
    w1cb[:, 6] = b1.astype(np.float16)

    # WU [L, 128, 2048]: 16 lhsT blocks of 128 cols:
    #   [w2(k2,j2) x4][u1a x4][u1b x4][u2 x4], block index = k2*2+j2
    wu = np.zeros((L, 128, 2048), f32)
    def put(off, m):
        for k2 in range(2):
            for j2 in range(2):
                blk = off + (k2 * 2 + j2) * 128
                wu[:, :, blk:blk + 128] = \
                    m[:, k2 * 128:(k2 + 1) * 128, j2 * 128:(j2 + 1) * 128]
    put(0, w2)
    put(512, u1[:, 0:256, :])
    put(1024, u1[:, 256:512, :])
    put(1536, u2)

    # BIAS [L, 1, 1024]: b2 | ub1 | ub2 | pad
    bias = np.zeros((L, 1, 1024), np.float16)
    bias[:, 0, 0:256] = b2
    bias[:, 0, 256:512] = ub1
    bias[:, 0, 512:768] = ub2

    embw = np.zeros((64, 256), f32)
    embw[:NATOM] = np.asarray(inp["embed_w"], f32)
    embb = np.asarray(inp["embed_b"], np.float16).reshape(1, 256)

    rw1 = np.asarray(inp["r_w1"], f32)
    rw1p = np.zeros((128, 512), f32)  # lhsT blocks (k2*2+j2)*128
    for k2 in range(2):
        for j2 in range(2):
            rw1p[:, (k2 * 2 + j2) * 128:(k2 * 2 + j2 + 1) * 128] = \
                rw1[k2 * 128:(k2 + 1) * 128, j2 * 128:(j2 + 1) * 128]
    rb1 = np.asarray(inp["r_b1"], np.float16).reshape(1, 256)
    rw2 = np.asarray(inp["r_w2"], f32)  # [256, 128]
    rw2p = np.zeros((128, 256), f32)
    for k2 in range(2):
        rw2p[:, k2 * 128:(k2 + 1) * 128] = rw2[k2 * 128:(k2 + 1) * 128, :]
    rb2 = np.asarray(inp["r_b2"], np.float16).reshape(1, 128)
    rw3 = np.asarray(inp["r_w3"], np.float16).reshape(128, 1)

    return dict(w1ab=w1ab, w1cb=w1cb, wu=wu, bias=bias, embw=embw, embb=embb,
                rw1=rw1p, rb1=rb1, rw2=rw2p, rb2=rb2, rw3=rw3)


# ----------------------------------------------------------------------------
# Device program
# ----------------------------------------------------------------------------
def _build(nwin: int, dump: bool = False, nlayers: int = NLAYERS,
           skip_edge: bool = False, skip_gather: bool = False,
           skip_coll: bool = False, reps: int = 1):
    import concourse.bacc as bacc
    import concourse.bass as cbass
    import concourse.mybir as mybir
    import concourse.tile as tile
    from concourse.masks import make_identity

    dt = mybir.dt
    AF = mybir.ActivationFunctionType
    ALU = mybir.AluOpType

    slots = nwin * WN
    tiles = nwin * WT
    ep = tiles * 128
    S = slots
    assert S % 256 == 0, S
    chunks = _chunk_list(S)  # (col0, width) node chunks, width 512 or 256
    NCH = len(chunks)
    MT = S // 128           # 128-row M-tiles for P

    nc = bacc.Bacc("TRN2", target_bir_lowering=False, debug=False,
                   num_swdge_queues=4)

    # ---- I/O ----
    comb_in = nc.dram_tensor("comb", [72, ep], dt.float16,
                             kind="ExternalInput")
    scat_in = nc.dram_tensor("scat", [128, tiles, WN], dt.float8e4,
                             kind="ExternalInput")
    srcw_in = nc.dram_tensor("srcw", [128, ep // 16], dt.int16,
                             kind="ExternalInput")
    af_in = nc.dram_tensor("af", [64, S], dt.float32r, kind="ExternalInput")
    deg_in = nc.dram_tensor("deg", [1, S], dt.float16, kind="ExternalInput")
    mask_in = nc.dram_tensor("mask", [1, S], dt.float16, kind="ExternalInput")
    w1ab_in = nc.dram_tensor("w1ab", [NLAYERS, 128, 1024], dt.float32r,
                             kind="ExternalInput")
    w1cb_in = nc.dram_tensor("w1cb", [NLAYERS, 8, 256], dt.float16,
                             kind="ExternalInput")
    wu_in = nc.dram_tensor("wu", [NLAYERS, 128, 2048], dt.float32r,
                           kind="ExternalInput")
    bias_in = nc.dram_tensor("bias", [NLAYERS, 1, 1024], dt.float16,
                             kind="ExternalInput")
    embw_in = nc.dram_tensor("embw", [64, 256], dt.float32r,
                             kind="ExternalInput")
    embb_in = nc.dram_tensor("embb", [1, 256], dt.float16,
                             kind="ExternalInput")
    rw1_in = nc.dram_tensor("rw1", [128, 512], dt.float32r,
                            kind="ExternalInput")
    rb1_in = nc.dram_tensor("rb1", [1, 256], dt.float16, kind="ExternalInput")
    rw2_in = nc.dram_tensor("rw2", [128, 256], dt.float32r,
                            kind="ExternalInput")
    rb2_in = nc.dram_tensor("rb2", [1, 128], dt.float16, kind="ExternalInput")
    rw3_in = nc.dram_tensor("rw3", [128, 1], dt.float16, kind="ExternalInput")

    partial_out = nc.dram_tensor("partial", [1, 1], dt.float32,
                                 kind="ExternalOutput")
    if dump:
        xdump_out = nc.dram_tensor("xdump", [NLAYERS + 1, 128, 2 * S],
                                   dt.float32, kind="ExternalOutput")

    # internal DRAM (double-buffered by layer parity so next-layer P writes
    # and AllGathers never WAR-conflict with current-layer reads)
    pdt = dt.float8e4 if PFP8 else dt.float16
    p_mine_ab = [nc.dram_tensor(f"p_mine{i}", [S, HID], pdt)
                 for i in range(2)]
    p_full_ab = [nc.dram_tensor(f"p_full{i}", [N_CORES * S, HID], pdt,
                                addr_space="Shared") for i in range(2)]

    with tile.TileContext(nc) as tc:
        with (
            tc.tile_pool(name="const", bufs=1) as cpool,
            tc.tile_pool(name="state", bufs=1) as spool,
            tc.tile_pool(name="wstage", bufs=2) as wpool,
            tc.tile_pool(name="gather", bufs=4) as gpool,
            tc.tile_pool(name="ework", bufs=3) as epool,
            tc.tile_pool(name="nwork", bufs=2) as npool,
            tc.tile_pool(name="epsum", bufs=3, space="PSUM") as epsum,
            tc.tile_pool(name="rpsum", bufs=2, space="PSUM") as rpsum,
            tc.tile_pool(name="npsum", bufs=3, space="PSUM") as npsum,
        ):
            # ---- resident constants ----
            comb_sb = cpool.tile([72, ep], dt.float16, tag="comb")
            nc.sync.dma_start(out=comb_sb[:], in_=comb_in[:])
            scat_sb = cpool.tile([128, tiles, WN], dt.float8e4, tag="scat")
            nc.sync.dma_start(out=scat_sb[:], in_=scat_in[:])
            srcw_sb = cpool.tile([128, ep // 16], dt.int16, tag="srcw")
            nc.sync.dma_start(out=srcw_sb[:], in_=srcw_in[:])
            af_sb = cpool.tile([64, S], dt.float32r, tag="af")
            nc.sync.dma_start(out=af_sb[:], in_=af_in[:])
            deg_sb = cpool.tile([1, S], dt.float16, tag="deg")
            nc.sync.dma_start(out=deg_sb[:], in_=deg_in[:])
            mask_sb = cpool.tile([1, S], dt.float16, tag="mask")
            nc.sync.dma_start(out=mask_sb[:], in_=mask_in[:])
            embw_sb = cpool.tile([64, 256], dt.float32r, tag="embw")
            nc.sync.dma_start(out=embw_sb[:], in_=embw_in[:])
            embb_sb = cpool.tile([1, 256], dt.float16, tag="embb")
            nc.sync.dma_start(out=embb_sb[:], in_=embb_in[:])
            rw1_sb = cpool.tile([128, 512], dt.float32r, tag="rw1")
            nc.sync.dma_start(out=rw1_sb[:], in_=rw1_in[:])
            rb1_sb = cpool.tile([1, 256], dt.float16, tag="rb1")
            nc.sync.dma_start(out=rb1_sb[:], in_=rb1_in[:])
            rw2_sb = cpool.tile([128, 256], dt.float32r, tag="rw2")
            nc.sync.dma_start(out=rw2_sb[:], in_=rw2_in[:])
            rb2_sb = cpool.tile([1, 128], dt.float16, tag="rb2")
            nc.sync.dma_start(out=rb2_sb[:], in_=rb2_in[:])
            rw3_sb = cpool.tile([128, 1], dt.float16, tag="rw3")
            nc.sync.dma_start(out=rw3_sb[:], in_=rw3_in[:])
            ident = cpool.tile([128, 128], dt.float32, tag="ident")
            make_identity(nc, ident[:])
            ident16 = cpool.tile([128, 128], dt.float16, tag="ident16")
            nc.vector.tensor_copy(out=ident16[:], in_=ident[:])
            assert PFP8, "DoubleRow scatter path requires fp8"
            identp = cpool.tile([128, 128], pdt, tag="identp")
            nc.vector.tensor_copy(out=identp[:], in_=ident[:])

            # ---- persistent state (feature-major, col = k2*S + slot) ----
            x_a = spool.tile([128, 2 * S], dt.float32r, tag="x_a")
            x_b = spool.tile([128, 2 * S], dt.float32r, tag="x_b")
            rh_fm = spool.tile([128, 2 * S], dt.float32r, tag="rh_fm")
            agg_fm = spool.tile([128, 2 * S], dt.float32r, tag="agg_fm")
            h_fm = spool.tile([128, 2 * S], dt.float32r, tag="h_fm")
            wr_a = spool.tile([72, nwin * 256], dt.float16, tag="wr_a")
            x_ab = [x_a, x_b]
            wr_ab = [wr_a, wr_a]

            def fm(t, k2, c0, n):
                return t[:, k2 * S + c0:k2 * S + c0 + n]

            # chunk ci completes once window (cc+cw)//WN - 1 is evicted
            LAG = int(os.environ.get("KERNEL_LAG", "0"))
            wend_chunks = {}
            for ci, (cc, cw) in enumerate(chunks):
                wend = min((cc + cw) // WN - 1 + LAG, nwin - 1)
                wend_chunks.setdefault(wend, []).append(ci)

            def stage_weights(layer):
                w1ab_sb = wpool.tile([128, 1024], dt.float32r, tag="w1ab")
                nc.sync.dma_start(out=w1ab_sb[:], in_=w1ab_in[layer])
                wu_sb = wpool.tile([128, 2048], dt.float32r, tag="wu")
                nc.sync.dma_start(out=wu_sb[:], in_=wu_in[layer])
                bias_sb = wpool.tile([1, 1024], dt.float16, tag="bias")
                nc.sync.dma_start(out=bias_sb[:], in_=bias_in[layer])
                return w1ab_sb, wu_sb, bias_sb

            def pq_chunk(layer, cc, cw, x_src, w1ab_sb, wrhs):
                """P rows (to p_mine) and Q windows (to wrhs) for slots
                [cc, cc+cw) of `layer`, plus that layer's W1c|b1 rows."""
                p_mine = p_mine_ab[layer % 2]
                for m in range(cc // 128, (cc + cw) // 128):
                    ps = npsum.tile([128, 512], dt.float32, tag="nps")
                    for k2 in range(2):
                        nc.tensor.matmul(
                            out=ps[:, 0:256],
                            lhsT=x_src[:, k2 * S + m * 128:
                                       k2 * S + (m + 1) * 128],
                            rhs=w1ab_sb[:, k2 * 512:k2 * 512 + 256],
                            start=(k2 == 0), stop=(k2 == 1))
                    p16 = npool.tile([128, 256], pdt, tag="p16")
                    nc.scalar.copy(out=p16[:], in_=ps[:, 0:256])
                    nc.sync.dma_start(
                        out=p_mine[m * 128:(m + 1) * 128, :], in_=p16[:])
                for w in range(cc // WN, (cc + cw) // WN):
                    qs = rpsum.tile([64, 256], dt.float32, tag="rps")
                    for k2 in range(2):
                        nc.tensor.matmul(
                            out=qs[:],
                            lhsT=x_src[:, k2 * S + w * WN:
                                       k2 * S + (w + 1) * WN],
                            rhs=w1ab_sb[:, k2 * 512 + 256:(k2 + 1) * 512],
                            start=(k2 == 0), stop=(k2 == 1))
                    nc.vector.tensor_copy(
                        out=wrhs[0:64, w * 256:(w + 1) * 256], in_=qs[:])
                    nc.sync.dma_start(
                        out=wrhs[64:72, w * 256:(w + 1) * 256],
                        in_=w1cb_in[layer])

            def node_chunk(layer, cc, cw, x_cur, x_nxt, wu_sb, bias_sb):
                """agg -> h -> x_new for slots [cc, cc+cw)."""
                for j2 in range(2):
                    ps = npsum.tile([128, 512], dt.float32, tag="nps")
                    for k2 in range(2):
                        blk = (k2 * 2 + j2) * 128
                        nc.tensor.matmul(
                            out=ps[:, :cw], lhsT=wu_sb[:, blk:blk + 128],
                            rhs=fm(rh_fm, k2, cc, cw),
                            start=(k2 == 0), stop=False)
                    nc.tensor.matmul(
                        out=ps[:, :cw],
                        lhsT=bias_sb[:, j2 * 128:(j2 + 1) * 128],
                        rhs=deg_sb[:, cc:cc + cw],
                        start=False, stop=True)
                    nc.scalar.copy(out=fm(agg_fm, j2, cc, cw), in_=ps[:, :cw])
                for j2 in range(2):
                    ps = npsum.tile([128, 512], dt.float32, tag="nps")
                    for k2 in range(2):
                        blk = 512 + (k2 * 2 + j2) * 128
                        nc.tensor.matmul(
                            out=ps[:, :cw], lhsT=wu_sb[:, blk:blk + 128],
                            rhs=fm(x_cur, k2, cc, cw),
                            start=(k2 == 0), stop=False)
                    for k2 in range(2):
                        blk = 1024 + (k2 * 2 + j2) * 128
                        nc.tensor.matmul(
                            out=ps[:, :cw], lhsT=wu_sb[:, blk:blk + 128],
                            rhs=fm(agg_fm, k2, cc, cw),
                            start=False, stop=False)
                    nc.tensor.matmul(
                        out=ps[:, :cw],
                        lhsT=bias_sb[:, 256 + j2 * 128:256 + (j2 + 1) * 128],
                        rhs=mask_sb[:, cc:cc + cw],
                        start=False, stop=True)
                    nc.scalar.activation(
                        fm(h_fm, j2, cc, cw), ps[:, :cw], AF.Relu)
                for j2 in range(2):
                    ps = npsum.tile([128, 512], dt.float32, tag="nps")
                    for k2 in range(2):
                        blk = 1536 + (k2 * 2 + j2) * 128
                        nc.tensor.matmul(
                            out=ps[:, :cw], lhsT=wu_sb[:, blk:blk + 128],
                            rhs=fm(h_fm, k2, cc, cw),
                            start=(k2 == 0), stop=False)
                    nc.tensor.matmul(
                        out=ps[:, :cw],
                        lhsT=bias_sb[:, 512 + j2 * 128:512 + (j2 + 1) * 128],
                        rhs=mask_sb[:, cc:cc + cw],
                        start=False, stop=True)
                    nc.scalar.copy(out=fm(x_nxt, j2, cc, cw), in_=ps[:, :cw])
                    if dump:
                        xd = npool.tile([128, 512], dt.float32, tag="xd")
                        nc.vector.tensor_copy(out=xd[:, :cw], in_=ps[:, :cw])
                        nc.sync.dma_start(
                            out=xdump_out[layer + 1, :,
                                          j2 * S + cc:j2 * S + cc + cw],
                            in_=xd[:, :cw])

            def readout_chunk(ci, cc, cw, x_fin, vred):
                """h1 -> h2 -> v partial for slots [cc, cc+cw)."""
                for j2 in range(2):
                    ps = npsum.tile([128, 512], dt.float32, tag="nps")
                    for k2 in range(2):
                        blk = (k2 * 2 + j2) * 128
                        nc.tensor.matmul(
                            out=ps[:, :cw], lhsT=rw1_sb[:, blk:blk + 128],
                            rhs=fm(x_fin, k2, cc, cw),
                            start=(k2 == 0), stop=False)
                    nc.tensor.matmul(
                        out=ps[:, :cw],
                        lhsT=rb1_sb[:, j2 * 128:(j2 + 1) * 128],
                        rhs=mask_sb[:, cc:cc + cw],
                        start=False, stop=True)
                    nc.scalar.activation(
                        fm(rh_fm, j2, cc, cw), ps[:, :cw], AF.Relu)
                ps = npsum.tile([128, 512], dt.float32, tag="nps")
                for k2 in range(2):
                    nc.tensor.matmul(
                        out=ps[:, :cw],
                        lhsT=rw2_sb[:, k2 * 128:(k2 + 1) * 128],
                        rhs=fm(rh_fm, k2, cc, cw),
                        start=(k2 == 0), stop=False)
                nc.tensor.matmul(
                    out=ps[:, :cw], lhsT=rb2_sb[:],
                    rhs=mask_sb[:, cc:cc + cw],
                    start=False, stop=True)
                h2 = npool.tile([128, 512], dt.float16, tag="h2")
                nc.scalar.activation(h2[:, :cw], ps[:, :cw], AF.Relu)
                vp = npsum.tile([128, 512], dt.float32, tag="nps")
                nc.tensor.matmul(
                    out=vp[0:1, :cw], lhsT=rw3_sb[:], rhs=h2[:, :cw],
                    start=True, stop=True)
                nc.vector.tensor_reduce(
                    out=vred[:, ci:ci + 1], in_=vp[0:1, :cw],
                    axis=mybir.AxisListType.X, op=ALU.add)

            def fire_ag(layer, ci):
                """Per-chunk AllGather of P, fired as soon as the chunk's
                p_mine rows are written (overlaps the remaining edge phase)."""
                if skip_coll:
                    return
                lp = layer % 2
                cc, cw = chunks[ci]
                nc.gpsimd.collective_compute(
                    "AllGather", ALU.bypass,
                    replica_groups=[list(range(N_CORES))],
                    ins=[p_mine_ab[lp][cc:cc + cw, :]],
                    outs=[p_full_ab[lp][8 * cc:8 * (cc + cw), :]])

            def _emit_body():
                # ---- embed: x0 = af.T @ embw + mask*embb ----
                for j2 in range(2):
                    for cc, cw in chunks:
                        ps = npsum.tile([128, 512], dt.float32, tag="nps")
                        nc.tensor.matmul(
                            out=ps[:, :cw],
                            lhsT=embw_sb[:, j2 * 128:(j2 + 1) * 128],
                            rhs=af_sb[:, cc:cc + cw],
                            start=True, stop=False)
                        nc.tensor.matmul(
                            out=ps[:, :cw],
                            lhsT=embb_sb[:, j2 * 128:(j2 + 1) * 128],
                            rhs=mask_sb[:, cc:cc + cw],
                            start=False, stop=True)
                        nc.scalar.copy(out=fm(x_a, j2, cc, cw),
                                       in_=ps[:, :cw])
                if dump:
                    for j2 in range(2):
                        for cc, cw in chunks:
                            x0_f32 = npool.tile([128, 512], dt.float32,
                                                tag="xd")
                            nc.vector.tensor_copy(
                                out=x0_f32[:, :cw], in_=fm(x_a, j2, cc, cw))
                            nc.sync.dma_start(
                                out=xdump_out[0, :,
                                              j2 * S + cc:j2 * S + cc + cw],
                                in_=x0_f32[:, :cw])

                # prologue: layer-0 weights + P/Q
                if nlayers > 0:
                    w_cur = stage_weights(0)
                    for ci, (cc, cw) in enumerate(chunks):
                        pq_chunk(0, cc, cw, x_a, w_cur[0], wr_ab[0])
                        if AGCHUNK:
                            fire_ag(0, ci)
                vred = npool.tile([1, NCH], dt.float32, tag="vred")

                for layer in range(nlayers):
                    x_cur = x_ab[layer % 2]
                    x_nxt = x_ab[(layer + 1) % 2]
                    wrhs = wr_ab[layer % 2]
                    wrhs_nxt = wr_ab[(layer + 1) % 2]
                    w1ab_sb, wu_sb, bias_sb = w_cur

                    # ---- AllGather P (non-chunked fallback) ----
                    if not skip_coll and not AGCHUNK:
                        lp = layer % 2
                        nc.gpsimd.collective_compute(
                            "AllGather", ALU.bypass,
                            replica_groups=[list(range(N_CORES))],
                            ins=[p_mine_ab[lp][:]], outs=[p_full_ab[lp][:]])

                    if layer + 1 < nlayers:
                        w_cur = stage_weights(layer + 1)

                    # ---- edge phase, node work interleaved per chunk ----
                    for chk in range(0 if skip_edge else nwin // 2):
                        pg = gpool.tile([128, 16, 256], pdt, tag="pg")
                        if skip_gather:
                            nc.gpsimd.memset(pg[:], 0)
                        else:
                            # GQ sub-gathers per chunk on distinct SWDGE
                            # queues (fewer = less Pool desc-gen overhead,
                            # more = finer first-use latency)
                            nsub = 16 // GQ
                            ncol = 128 // GQ
                            for hf in range(GQ):
                                nc.gpsimd.dma_gather(
                                    pg[:, hf * nsub:(hf + 1) * nsub, :],
                                    p_full_ab[layer % 2][:],
                                    srcw_sb[:, chk * 128 + hf * ncol:
                                            chk * 128 + (hf + 1) * ncol],
                                    ncol * 16, ncol * 16, 256,
                                    single_packet=False,
                                    queue_num=(GQ * chk + hf) % 4)
                        for wl in range(2):
                            w = chk * 2 + wl
                            rps = rpsum.tile([64, 256], dt.float32, tag="rps")
                            for t in range(WT):
                                g = w * WT + t
                                if t % 2 == 0:
                                    hps = epsum.tile([128, 512], dt.float32,
                                                     tag="hps")
                                    rr16 = epool.tile([128, 2, 256],
                                                      dt.float8e4, tag="r16")
                                hp = hps[:, (t % 2) * 256:(t % 2) * 256 + 256]
                                nc.tensor.matmul(
                                    out=hp,
                                    lhsT=comb_sb[:, g * 128:(g + 1) * 128],
                                    rhs=wrhs[:, w * 256:(w + 1) * 256],
                                    start=True, stop=False)
                                # h += I.T @ pg folds the P[src] add into
                                # PSUM on PE, freeing DVE per-tile work
                                nc.tensor.matmul(
                                    out=hp, lhsT=identp[:],
                                    rhs=pg[:, wl * WT + t, :],
                                    start=False, stop=True)
                                if t % 2 == 1:
                                    # one relu per tile-pair (alternating
                                    # engines), then one fp8 DoubleRow matmul
                                    # scatters both tiles into the window
                                    # accumulator
                                    rrf = rr16[:, 0:2, :]
                                    if t % 4 == 1:
                                        nc.scalar.activation(
                                            rrf, hps[:], AF.Relu)
                                    else:
                                        nc.vector.tensor_scalar_max(
                                            rrf, hps[:], 0.0)
                                    nc.tensor.matmul(
                                        out=rps[:],
                                        lhsT=scat_sb[:, g - 1:g + 1, :],
                                        rhs=rr16[:, 0:2, :],
                                        start=(t == 1), stop=(t == WT - 1),
                                        perf_mode=mybir.MatmulPerfMode
                                        .DoubleRow)
                            rrm = epool.tile([64, 256], dt.float32, tag="rrm")
                            nc.vector.tensor_copy(out=rrm[:], in_=rps[:])
                            for j2 in range(2):
                                tp = npsum.tile([128, 512], dt.float32,
                                                tag="nps")
                                nc.tensor.transpose(
                                    out=tp[:, 0:64],
                                    in_=rrm[:, j2 * 128:(j2 + 1) * 128],
                                    identity=ident[0:64, 0:64])
                                nc.scalar.copy(
                                    out=fm(rh_fm, j2, w * WN, WN),
                                    in_=tp[:, 0:64])
                            # interleaved node work for completed chunks
                            for ci in wend_chunks.get(w, []):
                                cc, cw = chunks[ci]
                                node_chunk(layer, cc, cw, x_cur, x_nxt,
                                           wu_sb, bias_sb)
                                if layer + 1 < nlayers:
                                    pq_chunk(layer + 1, cc, cw, x_nxt,
                                             w_cur[0], wrhs_nxt)
                                    if AGCHUNK:
                                        fire_ag(layer + 1, ci)
                                else:
                                    readout_chunk(ci, cc, cw, x_nxt, vred)
                    if skip_edge:
                        for ci, (cc, cw) in enumerate(chunks):
                            node_chunk(layer, cc, cw, x_cur, x_nxt,
                                       wu_sb, bias_sb)
                            if layer + 1 < nlayers:
                                pq_chunk(layer + 1, cc, cw, x_nxt,
                                         w_cur[0], wrhs_nxt)
                                if AGCHUNK:
                                    fire_ag(layer + 1, ci)
                            else:
                                readout_chunk(ci, cc, cw, x_nxt, vred)

                if nlayers == 0:
                    for ci, (cc, cw) in enumerate(chunks):
                        readout_chunk(ci, cc, cw, x_a, vred)

                psum_sb = npool.tile([1, 1], dt.float32, tag="psc")
                nc.vector.tensor_reduce(
                    out=psum_sb[:], in_=vred[:],
                    axis=mybir.AxisListType.X, op=ALU.add)
                nc.sync.dma_start(out=partial_out[:], in_=psum_sb[:])

            for _rep in range(reps):
                _emit_body()

    nc.compile()
    return nc


# ----------------------------------------------------------------------------
# Entry point
# ----------------------------------------------------------------------------
def kernel(**inputs) -> np.ndarray:
    from concourse.bass_utils import run_bass_kernel_spmd

    edge_index = np.asarray(inputs["edge_index"])
    plan = _plan(edge_index, np.asarray(inputs["edge_features"], np.float32),
                 np.asarray(inputs["atom_features"], np.float32))
    wts = _pack_weights(inputs)

    dump = bool(int(os.environ.get("KERNEL_DUMP", "0")))
    nlayers = int(os.environ.get("KERNEL_LAYERS", str(NLAYERS)))
    skip_edge = bool(int(os.environ.get("KERNEL_SKIP_EDGE", "0")))
    skip_gather = bool(int(os.environ.get("KERNEL_SKIP_GATHER", "0")))
    skip_coll = bool(int(os.environ.get("KERNEL_SKIP_COLL", "0")))
    key = (plan["nwin"], dump, nlayers, skip_edge, skip_gather, skip_coll,
           PFP8, GQ, AGCHUNK)
    if key not in _cache:
        import time as _t
        t0 = _t.time()
        _cache[key] = _build(plan["nwin"], dump=dump, nlayers=nlayers,
                             skip_edge=skip_edge, skip_gather=skip_gather,
                             skip_coll=skip_coll)
        print(f"build+schedule: {_t.time() - t0:.1f}s", flush=True)
    nc = _cache[key]

    shared = dict(w1ab=wts["w1ab"], w1cb=wts["w1cb"], wu=wts["wu"],
                  bias=wts["bias"], embw=wts["embw"], embb=wts["embb"],
                  rw1=wts["rw1"], rb1=wts["rb1"], rw2=wts["rw2"],
                  rb2=wts["rb2"], rw3=wts["rw3"])
    in_maps = []
    for c in range(N_CORES):
        pc = plan["per_core"][c]
        in_maps.append({**shared, "comb": pc["comb"], "scat": pc["scat"],
                        "srcw": pc["srcw"], "af": pc["af"], "deg": pc["deg"],
                        "mask": pc["mask"]})

    res = run_bass_kernel_spmd(nc, in_maps, list(range(N_CORES)))
    total = sum(float(res.results[c]["partial"][0, 0])
                for c in range(N_CORES))
    out = np.float32(total / N_NODES) + np.asarray(inputs["r_b3"],
                                                   np.float32).reshape(1)
    if dump:
        kernel._last_results = res  # type: ignore[attr-defined]
        kernel._last_plan = plan    # type: ignore[attr-defined]
    return out.astype(np.float32)



# revision 48
# speedup vs baseline: 1.0245x; 1.0245x over previous
"""Trainium2 Bass kernel for nn_BindingAffinityPredictor (GNN message passing).

Strategy (8 NeuronCores, SPMD):
- Sort edges by dst; partition nodes into 8 contiguous ranges with ~equal
  edge counts. Each core owns its node range and ALL edges into it, so the
  scatter-add is core-local.
- Within a core, nodes are greedily packed into windows of <=64 nodes and
  <=1024 edges; every window gets exactly 8 edge-tiles of 128 (padded).
  Node "slots" = window*64 + pos give a fixed, data-independent layout so
  one SPMD program serves all cores.
- Math trick: msg_in @ W1 splits into P[src] + Q[dst] + ef@W1c + b1 where
  P = x@W1a, Q = x@W1b are node-level matmuls (16x fewer FLOPs than
  per-edge). The second msg linear commutes with segment_sum:
  agg = segsum(relu(h1)) @ W2 + deg*b2.
- Edge phase is all fp8e4m3 (final output is a mean over 10k nodes, so
  quantization noise averages out; measured ~1.8e-3 rel err):
  * P table (x@W1a) cast to fp8, AllGather'ed (double-buffered by layer
    parity, optionally chunk-major + per-chunk early-fired collectives),
    then dma_gather'ed per edge (2 merged 1024-idx gathers per chunk).
  * Per edge tile [128 edges]: one fp16 matmul computes
    onehot_dst.T@(Q|W1c|b1), one fp8 matmul folds +I@P[src] into the same
    PSUM. One relu per tile-PAIR ([128,512] PSUM, alternating
    scalar/vector engines) writes fp8, then one fp8 DoubleRow scatter
    matmul (0.5 cyc/row, k-subtiles = the two scat one-hots / the two
    relu outputs) per pair accumulates both tiles' segment sums in PSUM.
- Node-level matmuls run in float32r (full PE speed at N>=256, ~fp32
  precision).
- PITFALL: every rhs partition row a matmul reads must be initialized
  (0 * NaN = NaN on PE can poison PSUM).
"""

import os
import sys

sys.path.insert(0, "/opt/trn_rl_repo")

import numpy as np

N_NODES = 10000
N_EDGES = 160000
HID = 256
NBOND = 6
NATOM = 62
NLAYERS = 6
N_CORES = 8

WN = 64          # nodes per window
WT = 8           # edge tiles per window
WE = WT * 128    # padded edge slots per window

# P-table distribution knobs (must match between _plan and _build)
PFP8 = bool(int(os.environ.get("KERNEL_PFP8", "1")))     # fp8e4m3 P table
GQ = int(os.environ.get("KERNEL_GQ", "2"))               # gathers per chunk
AGCHUNK = bool(int(os.environ.get("KERNEL_AGCHUNK", "1")))  # chunked AllGather

_cache: dict = {}


def _chunk_list(slots):
    # 512-wide chunks (last one 256): narrower chunks drop the float32r
    # node matmuls below the 256-row full-speed threshold — do not shrink
    chunks = []
    c0 = 0
    while c0 < slots:
        w = min(512, slots - c0)
        chunks.append((c0, w))
        c0 += w
    return chunks


# ----------------------------------------------------------------------------
# Host-side planning: core boundaries, windows, packed per-core arrays
# ----------------------------------------------------------------------------
def _plan(edge_index: np.ndarray, edge_features: np.ndarray,
          atom_features: np.ndarray):
    src = np.asarray(edge_index[0], dtype=np.int64)
    dst = np.asarray(edge_index[1], dtype=np.int64)
    deg = np.bincount(dst, minlength=N_NODES).astype(np.int64)
    cumdeg = np.concatenate([[0], np.cumsum(deg)])  # edges with dst < n

    bounds = [0]
    for c in range(1, N_CORES):
        bounds.append(int(np.searchsorted(cumdeg, N_EDGES * c / N_CORES)))
    bounds.append(N_NODES)

    order = np.argsort(dst, kind="stable")

    # greedy windows per core
    core_windows = []  # list of list of (node_start, node_cnt, edge_cnt)
    for c in range(N_CORES):
        wins = []
        n = bounds[c]
        while n < bounds[c + 1]:
            start, cnt, ecnt = n, 0, 0
            while (n < bounds[c + 1] and cnt < WN
                   and ecnt + deg[n] <= WE):
                ecnt += deg[n]
                cnt += 1
                n += 1
            assert cnt > 0, "single node exceeds window edge capacity"
            wins.append((start, cnt, ecnt))
        core_windows.append(wins)

    nwin = max(len(w) for w in core_windows)
    nwin = (nwin + 3) // 4 * 4  # S % 256 == 0 and even gather chunks
    slots = nwin * WN
    tiles = nwin * WT
    ep = tiles * 128  # padded edge slots per core

    # slot maps
    slot_of = np.full(N_NODES, -1, dtype=np.int64)
    core_of = np.full(N_NODES, -1, dtype=np.int64)
    for c, wins in enumerate(core_windows):
        for w, (s, cnt, _e) in enumerate(wins):
            slot_of[s:s + cnt] = w * WN + np.arange(cnt)
            core_of[s:s + cnt] = c
    if AGCHUNK:
        # chunk-major p_full layout: the AllGather for slot-chunk (cc, cw)
        # concatenates the 8 cores' rows contiguously at 8*cc, so row =
        # 8*cc + core*cw + (slot - cc)
        gslot = np.full(N_NODES, -1, dtype=np.int64)
        valid = slot_of >= 0
        for cc, cw in _chunk_list(slots):
            m = valid & (slot_of >= cc) & (slot_of < cc + cw)
            gslot[m] = 8 * cc + core_of[m] * cw + (slot_of[m] - cc)
    else:
        gslot = core_of * slots + slot_of  # global padded slot id

    import ml_dtypes
    f8 = ml_dtypes.float8_e4m3fn

    per_core = []
    for c, wins in enumerate(core_windows):
        comb = np.zeros((72, ep), np.float16)
        scat = np.zeros((128, tiles * WN), f8)
        srcg = np.zeros(ep, np.int16)
        for w, (s, cnt, ecnt) in enumerate(wins):
            if ecnt == 0:
                continue
            e_ids = order[cumdeg[s]:cumdeg[s + cnt]]  # dst-sorted, in window
            # sort window's edges by src for gather locality (dst stays in
            # window; one-hots are per-edge so order within window is free)
            e_ids = e_ids[np.argsort(gslot[src[e_ids]], kind="stable")]
            j = np.arange(ecnt)
            base = w * WE
            pos = base + j
            dl = dst[e_ids] - s                      # local node pos 0..cnt-1
            comb[0:64][dl, pos] = 1.0                 # bcast one-hot
            comb[64:70, pos] = edge_features[e_ids].T.astype(np.float16)
            comb[70, pos] = 1.0                       # bias ones
            tt = j // 128
            pp = j % 128
            scat[pp, (w * WT + tt) * WN + dl] = 1.0
            srcg[pos] = gslot[src[e_ids]].astype(np.int16)
        # wrap indices for dma_gather: element i at [i % 16, i // 16],
        # replicated across the 8 groups of 16 partitions
        wrapped = srcg.reshape(-1, 16).T              # [16, ep/16]
        srcw = np.tile(wrapped, (8, 1))               # [128, ep/16]

        af = np.zeros((64, slots), np.float32)
        degs = np.zeros((1, slots), np.float16)
        mask = np.zeros((1, slots), np.float16)
        for w, (s, cnt, _e) in enumerate(wins):
            sl = slice(w * WN, w * WN + cnt)
            af[:NATOM, sl] = atom_features[s:s + cnt].T
            degs[0, sl] = deg[s:s + cnt]
            mask[0, sl] = 1.0
        per_core.append(dict(comb=comb, scat=scat.reshape(128, tiles, WN),
                             srcw=srcw, af=af, deg=degs, mask=mask))

    return dict(nwin=nwin, slots=slots, tiles=tiles, ep=ep,
                per_core=per_core, bounds=bounds, core_windows=core_windows)


def _pack_weights(inp):
    """Pack weights into the layouts the device program consumes."""
    f32 = np.float32
    L = NLAYERS
    w1 = np.asarray(inp["msg_w1"], f32)   # [L, 518, 256]
    b1 = np.asarray(inp["msg_b1"], f32)   # [L, 256]
    w2 = np.asarray(inp["msg_w2"], f32)   # [L, 256, 256]
    b2 = np.asarray(inp["msg_b2"], f32)
    u1 = np.asarray(inp["upd_w1"], f32)   # [L, 512, 256]
    ub1 = np.asarray(inp["upd_b1"], f32)
    u2 = np.asarray(inp["upd_w2"], f32)
    ub2 = np.asarray(inp["upd_b2"], f32)

    # W1AB [L, 128, 1024]: col k2*512 + 0:256 = w1a, +256:512 = w1b
    w1ab = np.zeros((L, 128, 1024), f32)
    for k2 in range(2):
        w1ab[:, :, k2 * 512:k2 * 512 + 256] = w1[:, k2 * 128:(k2 + 1) * 128, :]
        w1ab[:, :, k2 * 512 + 256:(k2 + 1) * 512] = \
            w1[:, 256 + k2 * 128:256 + (k2 + 1) * 128, :]

    # W1CB [L, 8, 256] fp16: rows 0:6 = w1c, row 6 = b1, row 7 = 0
    w1cb = np.zeros((L, 8, 256), np.float16)
    w1cb[:, 0:6] = w1[:, 512:518, :].astype(np.float16)
    w1cb[:, 6] = b1.astype(np.float16)

    # WU [L, 128, 2048]: 16 lhsT blocks of 128 cols:
    #   [w2(k2,j2) x4][u1a x4][u1b x4][u2 x4], block index = k2*2+j2
    wu = np.zeros((L, 128, 2048), f32)
    def put(off, m):
        for k2 in range(2):
            for j2 in range(2):
                blk = off + (k2 * 2 + j2) * 128
                wu[:, :, blk:blk + 128] = \
                    m[:, k2 * 128:(k2 + 1) * 128, j2 * 128:(j2 + 1) * 128]
    put(0, w2)
    put(512, u1[:, 0:256, :])
    put(1024, u1[:, 256:512, :])
    put(1536, u2)

    # BIAS [L, 1, 1024]: b2 | ub1 | ub2 | pad
    bias = np.zeros((L, 1, 1024), np.float16)
    bias[:, 0, 0:256] = b2
    bias[:, 0, 256:512] = ub1
    bias[:, 0, 512:768] = ub2

    embw = np.zeros((64, 256), f32)
    embw[:NATOM] = np.asarray(inp["embed_w"], f32)
    embb = np.asarray(inp["embed_b"], np.float16).reshape(1, 256)

    rw1 = np.asarray(inp["r_w1"], f32)
    rw1p = np.zeros((128, 512), f32)  # lhsT blocks (k2*2+j2)*128
    for k2 in range(2):
        for j2 in range(2):
            rw1p[:, (k2 * 2 + j2) * 128:(k2 * 2 + j2 + 1) * 128] = \
                rw1[k2 * 128:(k2 + 1) * 128, j2 * 128:(j2 + 1) * 128]
    rb1 = np.asarray(inp["r_b1"], np.float16).reshape(1, 256)
    rw2 = np.asarray(inp["r_w2"], f32)  # [256, 128]
    rw2p = np.zeros((128, 256), f32)
    for k2 in range(2):
        rw2p[:, k2 * 128:(k2 + 1) * 128] = rw2[k2 * 128:(k2 + 1) * 128, :]
    rb2 = np.asarray(inp["r_b2"], np.float16).reshape(1, 128)
    rw3 = np.asarray(inp["r_w3"], np.float16).reshape(128, 1)

    return dict(w1ab=w1ab, w1cb=w1cb, wu=wu, bias=bias, embw=embw, embb=embb,
                rw1=rw1p, rb1=rb1, rw2=rw2p, rb2=rb2, rw3=rw3)


# ----------------------------------------------------------------------------
# Device program
# ----------------------------------------------------------------------------
def _build(nwin: int, dump: bool = False, nlayers: int = NLAYERS,
           skip_edge: bool = False, skip_gather: bool = False,
           skip_coll: bool = False, reps: int = 1):
    import concourse.bacc as bacc
    import concourse.bass as cbass
    import concourse.mybir as mybir
    import concourse.tile as tile
    from concourse.masks import make_identity

    dt = mybir.dt
    AF = mybir.ActivationFunctionType
    ALU = mybir.AluOpType

    slots = nwin * WN
    tiles = nwin * WT
    ep = tiles * 128
    S = slots
    assert S % 256 == 0, S
    chunks = _chunk_list(S)  # (col0, width) node chunks, width 512 or 256
    NCH = len(chunks)
    MT = S // 128           # 128-row M-tiles for P

    nc = bacc.Bacc("TRN2", target_bir_lowering=False, debug=False,
                   num_swdge_queues=4)

    # ---- I/O ----
    comb_in = nc.dram_tensor("comb", [72, ep], dt.float16,
                             kind="ExternalInput")
    scat_in = nc.dram_tensor("scat", [128, tiles, WN], dt.float8e4,
                             kind="ExternalInput")
    srcw_in = nc.dram_tensor("srcw", [128, ep // 16], dt.int16,
                             kind="ExternalInput")
    af_in = nc.dram_tensor("af", [64, S], dt.float32r, kind="ExternalInput")
    deg_in = nc.dram_tensor("deg", [1, S], dt.float16, kind="ExternalInput")
    mask_in = nc.dram_tensor("mask", [1, S], dt.float16, kind="ExternalInput")
    w1ab_in = nc.dram_tensor("w1ab", [NLAYERS, 128, 1024], dt.float32r,
                             kind="ExternalInput")
    w1cb_in = nc.dram_tensor("w1cb", [NLAYERS, 8, 256], dt.float16,
                             kind="ExternalInput")
    wu_in = nc.dram_tensor("wu", [NLAYERS, 128, 2048], dt.float32r,
                           kind="ExternalInput")
    bias_in = nc.dram_tensor("bias", [NLAYERS, 1, 1024], dt.float16,
                             kind="ExternalInput")
    embw_in = nc.dram_tensor("embw", [64, 256], dt.float32r,
                             kind="ExternalInput")
    embb_in = nc.dram_tensor("embb", [1, 256], dt.float16,
                             kind="ExternalInput")
    rw1_in = nc.dram_tensor("rw1", [128, 512], dt.float32r,
                            kind="ExternalInput")
    rb1_in = nc.dram_tensor("rb1", [1, 256], dt.float16, kind="ExternalInput")
    rw2_in = nc.dram_tensor("rw2", [128, 256], dt.float32r,
                            kind="ExternalInput")
    rb2_in = nc.dram_tensor("rb2", [1, 128], dt.float16, kind="ExternalInput")
    rw3_in = nc.dram_tensor("rw3", [128, 1], dt.float16, kind="ExternalInput")

    partial_out = nc.dram_tensor("partial", [1, 1], dt.float32,
                                 kind="ExternalOutput")
    if dump:
        xdump_out = nc.dram_tensor("xdump", [NLAYERS + 1, 128, 2 * S],
                                   dt.float32, kind="ExternalOutput")

    # internal DRAM (double-buffered by layer parity so next-layer P writes
    # and AllGathers never WAR-conflict with current-layer reads)
    pdt = dt.float8e4 if PFP8 else dt.float16
    p_mine_ab = [nc.dram_tensor(f"p_mine{i}", [S, HID], pdt)
                 for i in range(2)]
    p_full_ab = [nc.dram_tensor(f"p_full{i}", [N_CORES * S, HID], pdt,
                                addr_space="Shared") for i in range(2)]

    with tile.TileContext(nc) as tc:
        with (
            tc.tile_pool(name="const", bufs=1) as cpool,
            tc.tile_pool(name="state", bufs=1) as spool,
            tc.tile_pool(name="wstage", bufs=2) as wpool,
            tc.tile_pool(name="gather", bufs=4) as gpool,
            tc.tile_pool(name="ework", bufs=3) as epool,
            tc.tile_pool(name="nwork", bufs=2) as npool,
            tc.tile_pool(name="epsum", bufs=3, space="PSUM") as epsum,
            tc.tile_pool(name="rpsum", bufs=2, space="PSUM") as rpsum,
            tc.tile_pool(name="npsum", bufs=3, space="PSUM") as npsum,
        ):
            # ---- resident constants ----
            comb_sb = cpool.tile([72, ep], dt.float16, tag="comb")
            nc.sync.dma_start(out=comb_sb[:], in_=comb_in[:])
            scat_sb = cpool.tile([128, tiles, WN], dt.float8e4, tag="scat")
            nc.sync.dma_start(out=scat_sb[:], in_=scat_in[:])
            srcw_sb = cpool.tile([128, ep // 16], dt.int16, tag="srcw")
            nc.sync.dma_start(out=srcw_sb[:], in_=srcw_in[:])
            af_sb = cpool.tile([64, S], dt.float32r, tag="af")
            nc.sync.dma_start(out=af_sb[:], in_=af_in[:])
            deg_sb = cpool.tile([1, S], dt.float16, tag="deg")
            nc.sync.dma_start(out=deg_sb[:], in_=deg_in[:])
            mask_sb = cpool.tile([1, S], dt.float16, tag="mask")
            nc.sync.dma_start(out=mask_sb[:], in_=mask_in[:])
            embw_sb = cpool.tile([64, 256], dt.float32r, tag="embw")
            nc.sync.dma_start(out=embw_sb[:], in_=embw_in[:])
            embb_sb = cpool.tile([1, 256], dt.float16, tag="embb")
            nc.sync.dma_start(out=embb_sb[:], in_=embb_in[:])
            rw1_sb = cpool.tile([128, 512], dt.float32r, tag="rw1")
            nc.sync.dma_start(out=rw1_sb[:], in_=rw1_in[:])
            rb1_sb = cpool.tile([1, 256], dt.float16, tag="rb1")
            nc.sync.dma_start(out=rb1_sb[:], in_=rb1_in[:])
            rw2_sb = cpool.tile([128, 256], dt.float32r, tag="rw2")
            nc.sync.dma_start(out=rw2_sb[:], in_=rw2_in[:])
            rb2_sb = cpool.tile([1, 128], dt.float16, tag="rb2")
            nc.sync.dma_start(out=rb2_sb[:], in_=rb2_in[:])
            rw3_sb = cpool.tile([128, 1], dt.float16, tag="rw3")
            nc.sync.dma_start(out=rw3_sb[:], in_=rw3_in[:])
            ident = cpool.tile([128, 128], dt.float32, tag="ident")
            make_identity(nc, ident[:])
            ident16 = cpool.tile([128, 128], dt.float16, tag="ident16")
            nc.vector.tensor_copy(out=ident16[:], in_=ident[:])
            assert PFP8, "DoubleRow scatter path requires fp8"
            identp = cpool.tile([128, 128], pdt, tag="identp")
            nc.vector.tensor_copy(out=identp[:], in_=ident[:])

            # ---- persistent state (feature-major, col = k2*S + slot) ----
            x_a = spool.tile([128, 2 * S], dt.float32r, tag="x_a")
            x_b = spool.tile([128, 2 * S], dt.float32r, tag="x_b")
            rh_fm = spool.tile([128, 2 * S], dt.float32r, tag="rh_fm")
            agg_fm = spool.tile([128, 2 * S], dt.float32r, tag="agg_fm")
            h_fm = spool.tile([128, 2 * S], dt.float32r, tag="h_fm")
            wr_a = spool.tile([72, nwin * 256], dt.float16, tag="wr_a")
            x_ab = [x_a, x_b]
            wr_ab = [wr_a, wr_a]

            def fm(t, k2, c0, n):
                return t[:, k2 * S + c0:k2 * S + c0 + n]

            # chunk ci completes once window (cc+cw)//WN - 1 is evicted
            LAG = int(os.environ.get("KERNEL_LAG", "0"))
            wend_chunks = {}
            for ci, (cc, cw) in enumerate(chunks):
                wend = min((cc + cw) // WN - 1 + LAG, nwin - 1)
                wend_chunks.setdefault(wend, []).append(ci)

            def stage_weights(layer):
                w1ab_sb = wpool.tile([128, 1024], dt.float32r, tag="w1ab")
                nc.sync.dma_start(out=w1ab_sb[:], in_=w1ab_in[layer])
                wu_sb = wpool.tile([128, 2048], dt.float32r, tag="wu")
                nc.sync.dma_start(out=wu_sb[:], in_=wu_in[layer])
                bias_sb = wpool.tile([1, 1024], dt.float16, tag="bias")
                nc.sync.dma_start(out=bias_sb[:], in_=bias_in[layer])
                return w1ab_sb, wu_sb, bias_sb

            def pq_chunk(layer, cc, cw, x_src, w1ab_sb, wrhs):
                """P rows (to p_mine) and Q windows (to wrhs) for slots
                [cc, cc+cw) of `layer`, plus that layer's W1c|b1 rows."""
                p_mine = p_mine_ab[layer % 2]
                for m in range(cc // 128, (cc + cw) // 128):
                    ps = npsum.tile([128, 512], dt.float32, tag="nps")
                    for k2 in range(2):
                        nc.tensor.matmul(
                            out=ps[:, 0:256],
                            lhsT=x_src[:, k2 * S + m * 128:
                                       k2 * S + (m + 1) * 128],
                            rhs=w1ab_sb[:, k2 * 512:k2 * 512 + 256],
                            start=(k2 == 0), stop=(k2 == 1))
                    p16 = npool.tile([128, 256], pdt, tag="p16")
                    nc.scalar.copy(out=p16[:], in_=ps[:, 0:256])
                    nc.sync.dma_start(
                        out=p_mine[m * 128:(m + 1) * 128, :], in_=p16[:])
                for w in range(cc // WN, (cc + cw) // WN):
                    qs = rpsum.tile([64, 256], dt.float32, tag="rps")
                    for k2 in range(2):
                        nc.tensor.matmul(
                            out=qs[:],
                            lhsT=x_src[:, k2 * S + w * WN:
                                       k2 * S + (w + 1) * WN],
                            rhs=w1ab_sb[:, k2 * 512 + 256:(k2 + 1) * 512],
                            start=(k2 == 0), stop=(k2 == 1))
                    nc.vector.tensor_copy(
                        out=wrhs[0:64, w * 256:(w + 1) * 256], in_=qs[:])
                    nc.sync.dma_start(
                        out=wrhs[64:72, w * 256:(w + 1) * 256],
                        in_=w1cb_in[layer])

            def node_chunk(layer, cc, cw, x_cur, x_nxt, wu_sb, bias_sb):
                """agg -> h -> x_new for slots [cc, cc+cw)."""
                for j2 in range(2):
                    ps = npsum.tile([128, 512], dt.float32, tag="nps")
                    for k2 in range(2):
                        blk = (k2 * 2 + j2) * 128
                        nc.tensor.matmul(
                            out=ps[:, :cw], lhsT=wu_sb[:, blk:blk + 128],
                            rhs=fm(rh_fm, k2, cc, cw),
                            start=(k2 == 0), stop=False)
                    nc.tensor.matmul(
                        out=ps[:, :cw],
                        lhsT=bias_sb[:, j2 * 128:(j2 + 1) * 128],
                        rhs=deg_sb[:, cc:cc + cw],
                        start=False, stop=True)
                    nc.scalar.copy(out=fm(agg_fm, j2, cc, cw), in_=ps[:, :cw])
                for j2 in range(2):
                    ps = npsum.tile([128, 512], dt.float32, tag="nps")
                    for k2 in range(2):
                        blk = 512 + (k2 * 2 + j2) * 128
                        nc.tensor.matmul(
                            out=ps[:, :cw], lhsT=wu_sb[:, blk:blk + 128],
                            rhs=fm(x_cur, k2, cc, cw),
                            start=(k2 == 0), stop=False)
                    for k2 in range(2):
                        blk = 1024 + (k2 * 2 + j2) * 128
                        nc.tensor.matmul(
                            out=ps[:, :cw], lhsT=wu_sb[:, blk:blk + 128],
                            rhs=fm(agg_fm, k2, cc, cw),
                            start=False, stop=False)
                    nc.tensor.matmul(
                        out=ps[:, :cw],
                        lhsT=bias_sb[:, 256 + j2 * 128:256 + (j2 + 1) * 128],
                        rhs=mask_sb[:, cc:cc + cw],
                        start=False, stop=True)
                    nc.scalar.activation(
                        fm(h_fm, j2, cc, cw), ps[:, :cw], AF.Relu)
                for j2 in range(2):
                    ps = npsum.tile([128, 512], dt.float32, tag="nps")
                    for k2 in range(2):
                        blk = 1536 + (k2 * 2 + j2) * 128
                        nc.tensor.matmul(
                            out=ps[:, :cw], lhsT=wu_sb[:, blk:blk + 128],
                            rhs=fm(h_fm, k2, cc, cw),
                            start=(k2 == 0), stop=False)
                    nc.tensor.matmul(
                        out=ps[:, :cw],
                        lhsT=bias_sb[:, 512 + j2 * 128:512 + (j2 + 1) * 128],
                        rhs=mask_sb[:, cc:cc + cw],
                        start=False, stop=True)
                    nc.scalar.copy(out=fm(x_nxt, j2, cc, cw), in_=ps[:, :cw])
                    if dump:
                        xd = npool.tile([128, 512], dt.float32, tag="xd")
                        nc.vector.tensor_copy(out=xd[:, :cw], in_=ps[:, :cw])
                        nc.sync.dma_start(
                            out=xdump_out[layer + 1, :,
                                          j2 * S + cc:j2 * S + cc + cw],
                            in_=xd[:, :cw])

            def readout_chunk(ci, cc, cw, x_fin, vred):
                """h1 -> h2 -> v partial for slots [cc, cc+cw)."""
                for j2 in range(2):
                    ps = npsum.tile([128, 512], dt.float32, tag="nps")
                    for k2 in range(2):
                        blk = (k2 * 2 + j2) * 128
                        nc.tensor.matmul(
                            out=ps[:, :cw], lhsT=rw1_sb[:, blk:blk + 128],
                            rhs=fm(x_fin, k2, cc, cw),
                            start=(k2 == 0), stop=False)
                    nc.tensor.matmul(
                        out=ps[:, :cw],
                        lhsT=rb1_sb[:, j2 * 128:(j2 + 1) * 128],
                        rhs=mask_sb[:, cc:cc + cw],
                        start=False, stop=True)
                    nc.scalar.activation(
                        fm(rh_fm, j2, cc, cw), ps[:, :cw], AF.Relu)
                ps = npsum.tile([128, 512], dt.float32, tag="nps")
                for k2 in range(2):
                    nc.tensor.matmul(
                        out=ps[:, :cw],
                        lhsT=rw2_sb[:, k2 * 128:(k2 + 1) * 128],
                        rhs=fm(rh_fm, k2, cc, cw),
                        start=(k2 == 0), stop=False)
                nc.tensor.matmul(
                    out=ps[:, :cw], lhsT=rb2_sb[:],
                    rhs=mask_sb[:, cc:cc + cw],
                    start=False, stop=True)
                h2 = npool.tile([128, 512], dt.float16, tag="h2")
                nc.scalar.activation(h2[:, :cw], ps[:, :cw], AF.Relu)
                vp = npsum.tile([128, 512], dt.float32, tag="nps")
                nc.tensor.matmul(
                    out=vp[0:1, :cw], lhsT=rw3_sb[:], rhs=h2[:, :cw],
                    start=True, stop=True)
                nc.vector.tensor_reduce(
                    out=vred[:, ci:ci + 1], in_=vp[0:1, :cw],
                    axis=mybir.AxisListType.X, op=ALU.add)

            def fire_ag(layer, ci):
                """Per-chunk AllGather of P, fired as soon as the chunk's
                p_mine rows are written (overlaps the remaining edge phase)."""
                if skip_coll:
                    return
                lp = layer % 2
                cc, cw = chunks[ci]
                nc.gpsimd.collective_compute(
                    "AllGather", ALU.bypass,
                    replica_groups=[list(range(N_CORES))],
                    ins=[p_mine_ab[lp][cc:cc + cw, :]],
                    outs=[p_full_ab[lp][8 * cc:8 * (cc + cw), :]])

            def _emit_body():
                # ---- embed: x0 = af.T @ embw + mask*embb ----
                for j2 in range(2):
                    for cc, cw in chunks:
                        ps = npsum.tile([128, 512], dt.float32, tag="nps")
                        nc.tensor.matmul(
                            out=ps[:, :cw],
                            lhsT=embw_sb[:, j2 * 128:(j2 + 1) * 128],
                            rhs=af_sb[:, cc:cc + cw],
                            start=True, stop=False)
                        nc.tensor.matmul(
                            out=ps[:, :cw],
                            lhsT=embb_sb[:, j2 * 128:(j2 + 1) * 128],
                            rhs=mask_sb[:, cc:cc + cw],
                            start=False, stop=True)
                        nc.scalar.copy(out=fm(x_a, j2, cc, cw),
                                       in_=ps[:, :cw])
                if dump:
                    for j2 in range(2):
                        for cc, cw in chunks:
                            x0_f32 = npool.tile([128, 512], dt.float32,
                                                tag="xd")
                            nc.vector.tensor_copy(
                                out=x0_f32[:, :cw], in_=fm(x_a, j2, cc, cw))
                            nc.sync.dma_start(
                                out=xdump_out[0, :,
                                              j2 * S + cc:j2 * S + cc + cw],
                                in_=x0_f32[:, :cw])

                # prologue: layer-0 weights + P/Q
                if nlayers > 0:
                    w_cur = stage_weights(0)
                    for ci, (cc, cw) in enumerate(chunks):
                        pq_chunk(0, cc, cw, x_a, w_cur[0], wr_ab[0])
                        if AGCHUNK:
                            fire_ag(0, ci)
                vred = npool.tile([1, NCH], dt.float32, tag="vred")

                for layer in range(nlayers):
                    x_cur = x_ab[layer % 2]
                    x_nxt = x_ab[(layer + 1) % 2]
                    wrhs = wr_ab[layer % 2]
                    wrhs_nxt = wr_ab[(layer + 1) % 2]
                    w1ab_sb, wu_sb, bias_sb = w_cur

                    # ---- AllGather P (non-chunked fallback) ----
                    if not skip_coll and not AGCHUNK:
                        lp = layer % 2
                        nc.gpsimd.collective_compute(
                            "AllGather", ALU.bypass,
                            replica_groups=[list(range(N_CORES))],
                            ins=[p_mine_ab[lp][:]], outs=[p_full_ab[lp][:]])

                    if layer + 1 < nlayers:
                        w_cur = stage_weights(layer + 1)

                    # ---- edge phase, node work interleaved per chunk ----
                    for chk in range(0 if skip_edge else nwin // 2):
                        pg = gpool.tile([128, 16, 256], pdt, tag="pg")
                        if skip_gather:
                            nc.gpsimd.memset(pg[:], 0)
                        else:
                            # GQ sub-gathers per chunk on distinct SWDGE
                            # queues (fewer = less Pool desc-gen overhead,
                            # more = finer first-use latency)
                            nsub = 16 // GQ
                            ncol = 128 // GQ
                            for hf in range(GQ):
                                nc.gpsimd.dma_gather(
                                    pg[:, hf * nsub:(hf + 1) * nsub, :],
                                    p_full_ab[layer % 2][:],
                                    srcw_sb[:, chk * 128 + hf * ncol:
                                            chk * 128 + (hf + 1) * ncol],
                                    ncol * 16, ncol * 16, 256,
                                    single_packet=False,
                                    queue_num=(GQ * chk + hf) % 4)
                        for wl in range(2):
                            w = chk * 2 + wl
                            rps = rpsum.tile([64, 256], dt.float32, tag="rps")
                            for t in range(WT):
                                g = w * WT + t
                                if t % 2 == 0:
                                    hps = epsum.tile([128, 512], dt.float32,
                                                     tag="hps")
                                    rr16 = epool.tile([128, 2, 256],
                                                      dt.float8e4, tag="r16")
                                hp = hps[:, (t % 2) * 256:(t % 2) * 256 + 256]
                                nc.tensor.matmul(
                                    out=hp,
                                    lhsT=comb_sb[:, g * 128:(g + 1) * 128],
                                    rhs=wrhs[:, w * 256:(w + 1) * 256],
                                    start=True, stop=False)
                                # h += I.T @ pg folds the P[src] add into
                                # PSUM on PE, freeing DVE per-tile work
                                nc.tensor.matmul(
                                    out=hp, lhsT=identp[:],
                                    rhs=pg[:, wl * WT + t, :],
                                    start=False, stop=True)
                                if t % 2 == 1:
                                    # one relu per tile-pair (alternating
                                    # engines), then one fp8 DoubleRow matmul
                                    # scatters both tiles into the window
                                    # accumulator
                                    rrf = rr16[:, 0:2, :]
                                    if t % 4 == 1:
                                        nc.scalar.activation(
                                            rrf, hps[:], AF.Relu)
                                    else:
                                        nc.vector.tensor_scalar_max(
                                            rrf, hps[:], 0.0)
                                    nc.tensor.matmul(
                                        out=rps[:],
                                        lhsT=scat_sb[:, g - 1:g + 1, :],
                                        rhs=rr16[:, 0:2, :],
                                        start=(t == 1), stop=(t == WT - 1),
                                        perf_mode=mybir.MatmulPerfMode
                                        .DoubleRow)
                            rrm = epool.tile([64, 256], dt.float32, tag="rrm")
                            nc.vector.tensor_copy(out=rrm[:], in_=rps[:])
                            for j2 in range(2):
                                tp = npsum.tile([128, 512], dt.float32,
                                                tag="nps")
                                nc.tensor.transpose(
                                    out=tp[:, 0:64],
                                    in_=rrm[:, j2 * 128:(j2 + 1) * 128],
                                    identity=ident[0:64, 0:64])
                                nc.scalar.copy(
                                    out=fm(rh_fm, j2, w * WN, WN),
                                    in_=tp[:, 0:64])
                            # interleaved node work for completed chunks
                            for ci in wend_chunks.get(w, []):
                                cc, cw = chunks[ci]
                                node_chunk(layer, cc, cw, x_cur, x_nxt,
                                           wu_sb, bias_sb)
                                if layer + 1 < nlayers:
                                    pq_chunk(layer + 1, cc, cw, x_nxt,
                                             w_cur[0], wrhs_nxt)
                                    if AGCHUNK:
                                        fire_ag(layer + 1, ci)
                                else:
                                    readout_chunk(ci, cc, cw, x_nxt, vred)
                    if skip_edge:
                        for ci, (cc, cw) in enumerate(chunks):
                            node_chunk(layer, cc, cw, x_cur, x_nxt,
                                       wu_sb, bias_sb)
                            if layer + 1 < nlayers:
                                pq_chunk(layer + 1, cc, cw, x_nxt,
                                         w_cur[0], wrhs_nxt)
                                if AGCHUNK:
                                    fire_ag(layer + 1, ci)
                            else:
                                readout_chunk(ci, cc, cw, x_nxt, vred)

                if nlayers == 0:
                    for ci, (cc, cw) in enumerate(chunks):
                        readout_chunk(ci, cc, cw, x_a, vred)

                psum_sb = npool.tile([1, 1], dt.float32, tag="psc")
                nc.vector.tensor_reduce(
                    out=psum_sb[:], in_=vred[:],
                    axis=mybir.AxisListType.X, op=ALU.add)
                nc.sync.dma_start(out=partial_out[:], in_=psum_sb[:])

            for _rep in range(reps):
                _emit_body()

    nc.compile()
    return nc


# ----------------------------------------------------------------------------
# Entry point
# ----------------------------------------------------------------------------
def kernel(**inputs) -> np.ndarray:
    from concourse.bass_utils import run_bass_kernel_spmd

    edge_index = np.asarray(inputs["edge_index"])
    plan = _plan(edge_index, np.asarray(inputs["edge_features"], np.float32),
                 np.asarray(inputs["atom_features"], np.float32))
    wts = _pack_weights(inputs)

    dump = bool(int(os.environ.get("KERNEL_DUMP", "0")))
    nlayers = int(os.environ.get("KERNEL_LAYERS", str(NLAYERS)))
    skip_edge = bool(int(os.environ.get("KERNEL_SKIP_EDGE", "0")))
    skip_gather = bool(int(os.environ.get("KERNEL_SKIP_GATHER", "0")))
    skip_coll = bool(int(os.environ.get("KERNEL_SKIP_COLL", "0")))
    key = (plan["nwin"], dump, nlayers, skip_edge, skip_gather, skip_coll,
           PFP8, GQ, AGCHUNK)
    if key not in _cache:
        import time as _t
        t0 = _t.time()
        _cache[key] = _build(plan["nwin"], dump=dump, nlayers=nlayers,
                             skip_edge=skip_edge, skip_gather=skip_gather,
                             skip_coll=skip_coll)
        print(f"build+schedule: {_t.time() - t0:.1f}s", flush=True)
    nc = _cache[key]

    shared = dict(w1ab=wts["w1ab"], w1cb=wts["w1cb"], wu=wts["wu"],
                  bias=wts["bias"], embw=wts["embw"], embb=wts["embb"],
                  rw1=wts["rw1"], rb1=wts["rb1"], rw2=wts["rw2"],
                  rb2=wts["rb2"], rw3=wts["rw3"])
    in_maps = []
    for c in range(N_CORES):
        pc = plan["per_core"][c]
        in_maps.append({**shared, "comb": pc["comb"], "scat": pc["scat"],
                        "srcw": pc["srcw"], "af": pc["af"], "deg": pc["deg"],
                        "mask": pc["mask"]})

    res = run_bass_kernel_spmd(nc, in_maps, list(range(N_CORES)))
    total = sum(float(res.results[c]["partial"][0, 0])
                for c in range(N_CORES))
    out = np.float32(total / N_NODES) + np.asarray(inputs["r_b3"],
                                                   np.float32).reshape(1)
    if dump:
        kernel._last_results = res  # type: ignore[attr-defined]
        kernel._last_plan = plan    # type: ignore[attr-defined]
    return out.astype(np.float32)

